# revision 3
# baseline (speedup 1.0000x reference)
"""AnchorHead detection post-processing (multi-level sigmoid + per-level top-k
+ gather + delta decode) on 8 Trainium2 NeuronCores.

Sharding: data-parallel over batch. Each of the 8 cores processes one image:
it streams that image's 5 levels of cls logits (63MB) through SBUF and
computes m[anchor] = max over the 80 classes (the memory-bound reduction,
~98.5% of all input bytes). The tiny remainder (per-level top-500 selection
on the 196k maxes, the 500x80 score gather + sigmoid, and box decode) runs
on host.

Self-contained: hardcodes all shapes from the problem spec.
"""
import math
import numpy as np

import concourse.bass as bass
import concourse.mybir as mybir
from concourse.bass_utils import run_bass_kernel_spmd

# ---- problem constants (hardcoded from spec) ----
STRIDES = [8, 16, 32, 64, 128]
SIZES = [128, 64, 32, 16, 8]
A = 9
C = 80
B = 8
NMS_PRE = 500
IMG = 1024.0
MAX_RATIO = abs(math.log(16.0 / 1000.0))

# device-side m layouts: (partitions, free) per level
# L0: [128, 1152]  m0[p, a*128+f]  -> anchor (a, s=p*128+f)
# L1: [72, 512]    m1[a*8+o, f]    -> anchor (a, s=o*512+f)
# L2: [72, 128]    m2[a*8+o, f]    -> anchor (a, s=o*128+f)
# L3: [72, 32]     m3[a*8+o, f]    -> anchor (a, s=o*32+f)
# L4: [72, 8]      m4[a*8+o, f]    -> anchor (a, s=o*8+f)
M_SHAPES = [(128, 1152), (72, 512), (72, 128), (72, 32), (72, 8)]

_GRAPH = None


def _build_graph():
    nc = bass.Bass()
    cls_in = [
        nc.declare_dram_parameter(
            f"cls_score_{i}", [A * C, SIZES[i], SIZES[i]], mybir.dt.float32,
            isOutput=False)
        for i in range(5)
    ]
    m_out = [
        nc.declare_dram_parameter(f"m_{i}", list(M_SHAPES[i]), mybir.dt.float32,
                                  isOutput=True)
        for i in range(5)
    ]

    # chunk list: (kind, level, idx). L0 has 9 per-anchor chunks; L1 has 4
    # class-chunks of 20; L2-4 one chunk each. 16 chunks total.
    chunks = [("L0", 0, a) for a in range(A)]
    chunks += [("L1", 1, cc) for cc in range(4)]
    chunks += [("Lx", 2, 0), ("Lx", 3, 0), ("Lx", 4, 0)]
    NCHUNK = len(chunks)

    def src_level(l):
        return cls_in[l][:].rearrange("(a c) h w -> a c (h w)", a=A)

    with (
        nc.sbuf_tensor([128, 10240], mybir.dt.float32) as buf0,
        nc.sbuf_tensor([128, 10240], mybir.dt.float32) as buf1,
        nc.sbuf_tensor([128, 1152], mybir.dt.float32) as m0,
        nc.sbuf_tensor([72, 512], mybir.dt.float32) as m1,
        nc.sbuf_tensor([72, 512], mybir.dt.float32) as m1t,
        nc.sbuf_tensor([72, 128], mybir.dt.float32) as m2,
        nc.sbuf_tensor([72, 32], mybir.dt.float32) as m3,
        nc.sbuf_tensor([72, 8], mybir.dt.float32) as m4,
        nc.semaphore("dma_sem0") as dma_sem0,
        nc.semaphore("dma_sem1") as dma_sem1,
        nc.semaphore("out_sem") as out_sem,
        nc.semaphore("v_sem") as v_sem,
        nc.Block() as block,
    ):
        bufs = [buf0, buf1]
        m_sb = [m0, m1, m2, m3, m4]

        def tile_view(k, kind, l, i):
            b = bufs[k % 2]
            if kind == "L0":
                return b[:].rearrange("p (c f) -> p c f", c=C)          # [128,80,128]
            if kind == "L1":
                return b[:72, :].rearrange("p (c f) -> p c f", c=20)    # [72,20,512]
            hw = SIZES[l] * SIZES[l]
            f = hw // 8
            return b[:72, : C * f].rearrange("p (c f) -> p c f", c=C)   # [72,80,f]

        # per-buffer-parity DMA semaphores: chunk k's DMAs increment
        # dma_sem{k%2}. Within one parity, chunk k's DMAs are only issued
        # after the vector engine consumed chunk k-2 (v_sem gate), so the
        # cumulative per-parity count is race-free even though DMA
        # completions can reorder globally.
        ndma = [1 if kind == "L0" else A for kind, _, _ in chunks]
        cum_par = []  # cumulative DMA count on this chunk's parity sem
        tot = [0, 0]
        for k, n in enumerate(ndma):
            tot[k % 2] += n
            cum_par.append(tot[k % 2])
        dma_sems = [dma_sem0, dma_sem1]

        @block.sync
        def _(sync):
            for k, (kind, l, i) in enumerate(chunks):
                if k >= 2:
                    sync.wait_ge(v_sem, k - 1)
                tv = tile_view(k, kind, l, i)
                sem = dma_sems[k % 2]
                if kind == "L0":
                    s = src_level(0)[i]                       # [80, 16384]
                    s = s.rearrange("c (p f) -> p c f", f=128)  # [128,80,128]
                    sync.dma_start(out=tv, in_=s).then_inc(sem, 16)
                elif kind == "L1":
                    for a in range(A):
                        s = src_level(1)[a][i * 20:(i + 1) * 20]    # [20, 4096]
                        s = s.rearrange("c (o f) -> o c f", f=512)  # [8,20,512]
                        sync.dma_start(out=tv[a * 8:(a + 1) * 8], in_=s).then_inc(
                            sem, 16)
                else:
                    hw = SIZES[l] * SIZES[l]
                    f = hw // 8
                    for a in range(A):
                        s = src_level(l)[a]                        # [80, hw]
                        s = s.rearrange("c (o f) -> o c f", f=f)   # [8,80,f]
                        sync.dma_start(out=tv[a * 8:(a + 1) * 8], in_=s).then_inc(
                            sem, 16)
            sync.wait_ge(v_sem, NCHUNK)
            for l in range(5):
                sync.dma_start(out=m_out[l][:], in_=m_sb[l][:]).then_inc(out_sem, 16)
            sync.wait_ge(out_sem, 16 * 5)

        @block.vector
        def _(vector):
            for k, (kind, l, i) in enumerate(chunks):
                vector.wait_ge(dma_sems[k % 2], 16 * int(cum_par[k]))
                tv = tile_view(k, kind, l, i)
                red_view = tv.transpose([0, 2, 1])  # [p, f, c] -> innermost c
                if kind == "L0":
                    out = m0[:, i * 128:(i + 1) * 128]
                    vector.reduce_max(out, red_view,
                                      axis=mybir.AxisListType.X).then_inc(v_sem, 1)
                elif kind == "L1":
                    if i == 0:
                        vector.reduce_max(m1[:], red_view,
                                          axis=mybir.AxisListType.X).then_inc(v_sem, 1)
                    else:
                        vector.reduce_max(m1t[:], red_view,
                                          axis=mybir.AxisListType.X)
                        vector.tensor_max(m1[:], m1[:], m1t[:]).then_inc(v_sem, 1)
                else:
                    vector.reduce_max(m_sb[l][:], red_view,
                                      axis=mybir.AxisListType.X).then_inc(v_sem, 1)

    return nc


def _get_graph():
    global _GRAPH
    if _GRAPH is None:
        _GRAPH = _build_graph()
    return _GRAPH


# ---- host-side static tables ----

def _grid_anchors_np(stride, H, W):
    """mmdet AnchorGenerator, float64 then cast f32 (matches reference)."""
    base = stride * 4.0
    ratios = np.array([0.5, 1.0, 2.0], dtype=np.float64)
    scales = np.array([2 ** 0, 2 ** (1.0 / 3), 2 ** (2.0 / 3)], dtype=np.float64)
    h_ratios = np.sqrt(ratios)
    w_ratios = 1.0 / h_ratios
    ws = (base * w_ratios[:, None] * scales[None, :]).reshape(-1)
    hs = (base * h_ratios[:, None] * scales[None, :]).reshape(-1)
    base_anchors = np.stack([-ws / 2, -hs / 2, ws / 2, hs / 2], axis=1)
    sx = np.arange(W, dtype=np.float64) * stride
    sy = np.arange(H, dtype=np.float64) * stride
    yy, xx = np.meshgrid(sy, sx, indexing="ij")
    shifts = np.stack([xx, yy, xx, yy], axis=-1).reshape(-1, 1, 4)
    anchors = (shifts + base_anchors[None]).reshape(-1, 4)
    return anchors.astype(np.float32)


def _lin_index_map(l):
    """linear anchor index n for each element of the device m_{l} layout."""
    P, F = M_SHAPES[l]
    H = SIZES[l]
    HW = H * H
    if l == 0:
        p = np.arange(128)[:, None, None]
        a = np.arange(A)[None, :, None]
        f = np.arange(128)[None, None, :]
        s = p * 128 + f
        return ((s * A) + a).reshape(128, 1152)
    fcnt = HW // 8
    ao = np.arange(72)[:, None]
    a = ao // 8
    o = ao % 8
    f = np.arange(fcnt)[None, :]
    s = o * fcnt + f
    return (s * A) + a  # [72, fcnt]


_ANCHORS = [_grid_anchors_np(STRIDES[l], SIZES[l], SIZES[l]) for l in range(5)]
_LINMAP = [_lin_index_map(l) for l in range(5)]


def _decode_np(anchors, deltas):
    dx, dy = deltas[..., 0], deltas[..., 1]
    dw = np.clip(deltas[..., 2], -MAX_RATIO, MAX_RATIO).astype(np.float32)
    dh = np.clip(deltas[..., 3], -MAX_RATIO, MAX_RATIO).astype(np.float32)
    px = (anchors[..., 0] + anchors[..., 2]) * np.float32(0.5)
    py = (anchors[..., 1] + anchors[..., 3]) * np.float32(0.5)
    pw = anchors[..., 2] - anchors[..., 0]
    ph = anchors[..., 3] - anchors[..., 1]
    gx = px + pw * dx
    gy = py + ph * dy
    gw = pw * np.exp(dw)
    gh = ph * np.exp(dh)
    half = np.float32(0.5)
    x1 = np.clip(gx - gw * half, 0.0, IMG).astype(np.float32)
    y1 = np.clip(gy - gh * half, 0.0, IMG).astype(np.float32)
    x2 = np.clip(gx + gw * half, 0.0, IMG).astype(np.float32)
    y2 = np.clip(gy + gh * half, 0.0, IMG).astype(np.float32)
    return np.stack([x1, y1, x2, y2], axis=-1)


def _run_device(in_maps, **kw):
    nc = _get_graph()
    return run_bass_kernel_spmd(nc, in_maps, core_ids=list(range(B)), **kw)


def kernel(**inputs):
    cls = [np.asarray(inputs[f"cls_score_{l}"], dtype=np.float32) for l in range(5)]
    box = [np.asarray(inputs[f"bbox_pred_{l}"], dtype=np.float32) for l in range(5)]

    in_maps = [
        {f"cls_score_{l}": np.ascontiguousarray(cls[l][b]) for l in range(5)}
        for b in range(B)
    ]
    res = _run_device(in_maps).results

    bboxes = np.empty((B, 5 * NMS_PRE, 4), dtype=np.float32)
    scores = np.empty((B, 5 * NMS_PRE, C), dtype=np.float32)

    for b in range(B):
        col = 0
        for l in range(5):
            H = SIZES[l]
            HW = H * H
            m_dev = res[b][f"m_{l}"]
            vals = m_dev.ravel()
            lins = _LINMAP[l].ravel()
            # top-500 by value desc, ties -> lowest linear index (lax.top_k)
            if vals.size > 2 * NMS_PRE:
                part = np.argpartition(-vals, NMS_PRE + 16)[: NMS_PRE + 16]
            else:
                part = np.arange(vals.size)
            pv = vals[part]
            pl = lins[part]
            order = np.lexsort((pl, -pv))[:NMS_PRE]
            sel_lin = pl[order]  # [500] linear anchor idx, sorted desc by score

            a = sel_lin % A
            s = sel_lin // A
            h = s // H
            w = s % H
            # scores: sigmoid of the 80 logits of each selected anchor
            logit_rows = cls[l][b][(a[:, None] * C + np.arange(C)[None, :]), h[:, None], w[:, None]]
            scores[b, col:col + NMS_PRE] = 1.0 / (1.0 + np.exp(-logit_rows))
            # deltas + anchors -> decode
            d = box[l][b][(a[:, None] * 4 + np.arange(4)[None, :]), h[:, None], w[:, None]]
            anc = _ANCHORS[l][sel_lin]
            bboxes[b, col:col + NMS_PRE] = _decode_np(anc, d)
            col += NMS_PRE

    return bboxes, scores


# revision 6
# speedup vs baseline: 1.9663x; 1.9663x over previous
"""AnchorHead detection post-processing (multi-level sigmoid + per-level top-k
+ gather + delta decode) on 8 Trainium2 NeuronCores.

Sharding: data-parallel over batch. Each of the 8 cores processes one image:
it streams that image's 5 levels of cls logits (63MB) through SBUF and
computes m[anchor] = max over the 80 classes (the memory-bound reduction,
~98.5% of all input bytes). The tiny remainder (per-level top-500 selection
on the 196k maxes, the 500x80 score gather + sigmoid, and box decode) runs
on host.

Device pipeline per core:
  - L0 (128x128, 47MB): 9 per-anchor chunks DMA'd as [128 spatial-fold, 80
    classes, 128 spatial] (512B runs, all partitions -> all 16 DMA engines),
    class-max via a tensor_tensor max tree (80->40->20->10->5) + one small
    strided reduce. Avoids the 1.6x strided-innermost reduce penalty.
  - L1-L4 (15.7MB): channel-major DMA [120ch, spatial] (8-64KB runs),
    TensorE transpose of 128-column blocks into PSUM, vector reduce_max
    straight from PSUM (classes innermost), batched 2 blocks/op.
  - The two streams are interleaved so DMA, TensorE and Vector overlap.

Self-contained: hardcodes all shapes from the problem spec.
"""
import math
import numpy as np

import concourse.bass as bass
import concourse.mybir as mybir
from concourse.bass_utils import run_bass_kernel_spmd

# ---- problem constants (hardcoded from spec) ----
STRIDES = [8, 16, 32, 64, 128]
SIZES = [128, 64, 32, 16, 8]
A = 9
C = 80
B = 8
NMS_PRE = 500
IMG = 1024.0
MAX_RATIO = abs(math.log(16.0 / 1000.0))

# device-side m layouts: (partitions, free) per level
# L0: [128, 1152]  m0[p, a*128+f] -> anchor (a, s=p*128+f)
# L1-L4 (block-major): m[q, lb*9+a] -> anchor (a, s=lb*128+q)
M_SHAPES = [(128, 1152), (128, 288), (128, 72), (128, 18), (64, 9)]

_GRAPH = None


def _build_graph():
    nc = bass.Bass()
    cls_in = [
        nc.declare_dram_parameter(
            f"cls_score_{i}", [A * C, SIZES[i], SIZES[i]], mybir.dt.float32,
            isOutput=False)
        for i in range(5)
    ]
    m_out = [
        nc.declare_dram_parameter(f"m_{i}", list(M_SHAPES[i]), mybir.dt.float32,
                                  isOutput=True)
        for i in range(5)
    ]

    # small-level groups: (level, s0, S). S <= 1024 so 6 ch-tiles of
    # [120, S] double-buffer in SBUF.
    sgroups = [(1, 0, 1024), (1, 1024, 1024), (1, 2048, 1024), (1, 3072, 1024),
               (2, 0, 1024), (3, 0, 256), (4, 0, 64)]
    NSG = len(sgroups)
    # per-group 128-column transpose blocks
    sg_blocks = [max(1, S // 128) for _, _, S in sgroups]  # [8,8,8,8,8,2,1]
    NSB = sum(sg_blocks)  # 43

    # global small-block table: (group, block_in_group, level, level_block, width)
    blocks = []
    lvl_blk = {1: 0, 2: 0, 3: 0, 4: 0}
    for g, (l, s0, S) in enumerate(sgroups):
        for bi in range(sg_blocks[g]):
            w = min(128, S)
            blocks.append((g, bi, l, lvl_blk[l], w))
            lvl_blk[l] += 1

    # vector ops over small blocks: batch 2 consecutive (even-aligned) blocks
    # when both exist and share width 128; else single.
    vops = []  # list of lists of global block ids
    bid = 0
    while bid < NSB:
        if (bid + 1 < NSB and bid % 2 == 0
                and blocks[bid][4] == 128 and blocks[bid + 1][4] == 128
                and blocks[bid][2] == blocks[bid + 1][2]
                and blocks[bid + 1][3] == blocks[bid][3] + 1):
            vops.append([bid, bid + 1])
            bid += 2
        else:
            vops.append([bid])
            bid += 1
    # for PSUM rotation: number of vops fully covering blocks <= x
    op_end = [op[-1] for op in vops]

    def vs_need(b):
        # vector ops that must be done before writing pair b%4 for block b:
        # any op touching a block < b on this pair. Ops span <= 2 consecutive
        # blocks, so exactly the ops whose last block <= b-3 qualify (an op
        # whose first block is b-4 ends at <= b-3).
        x = b - 3
        return sum(1 for e in op_end if e <= x)

    # L0 chunks: 9 (one per anchor a)
    NL0 = 9

    from contextlib import ExitStack
    with ExitStack() as ctx:
        l0buf = ctx.enter_context(nc.sbuf_tensor([128, 2, C, 128], mybir.dt.float32))
        sc1 = ctx.enter_context(nc.sbuf_tensor([128, 5120], mybir.dt.float32))
        sc2 = ctx.enter_context(nc.sbuf_tensor([128, 2560], mybir.dt.float32))
        sbuf_s = ctx.enter_context(nc.sbuf_tensor([120, 2, 6, 1024], mybir.dt.float32))
        ident = ctx.enter_context(nc.sbuf_tensor([120, 120], mybir.dt.float32))
        m0 = ctx.enter_context(nc.sbuf_tensor([128, 1152], mybir.dt.float32))
        m1 = ctx.enter_context(nc.sbuf_tensor([128, 288], mybir.dt.float32))
        m2 = ctx.enter_context(nc.sbuf_tensor([128, 72], mybir.dt.float32))
        m3 = ctx.enter_context(nc.sbuf_tensor([128, 18], mybir.dt.float32))
        m4 = ctx.enter_context(nc.sbuf_tensor([64, 9], mybir.dt.float32))
        psA = ctx.enter_context(nc.psum_tensor([128, 4, 512], mybir.dt.float32))
        psB = ctx.enter_context(nc.psum_tensor([128, 4, 512], mybir.dt.float32))
        dma_l0_0 = ctx.enter_context(nc.semaphore("dma_l0_0"))
        dma_l0_1 = ctx.enter_context(nc.semaphore("dma_l0_1"))
        dma_s_0 = ctx.enter_context(nc.semaphore("dma_s_0"))
        dma_s_1 = ctx.enter_context(nc.semaphore("dma_s_1"))
        t_sem = ctx.enter_context(nc.semaphore("t_sem"))    # +1 per small block
        tg_sem = ctx.enter_context(nc.semaphore("tg_sem"))  # +1 per small group
        v0_sem = ctx.enter_context(nc.semaphore("v0_sem"))  # +1 per L0 chunk
        vs_sem = ctx.enter_context(nc.semaphore("vs_sem"))  # +1 per small vop
        init_sem = ctx.enter_context(nc.semaphore("init_sem"))
        out_sem = ctx.enter_context(nc.semaphore("out_sem"))
        block = ctx.enter_context(nc.Block())
        m_sb = [m0, m1, m2, m3, m4]
        dma_l0 = [dma_l0_0, dma_l0_1]
        dma_s = [dma_s_0, dma_s_1]

        # interleaved issue order: smalls group i before L0 chunk i
        seq = []
        for i in range(max(NSG, NL0)):
            if i < NSG:
                seq.append(("S", i))
            if i < NL0:
                seq.append(("L0", i))

        # cumulative DMA counts per parity
        s_par_cum = [0, 0]
        s_cum = []  # value to wait on for group g (on parity g%2)
        for g in range(NSG):
            s_par_cum[g % 2] += 6
            s_cum.append(s_par_cum[g % 2])
        l0_par_cum = [0, 0]
        l0_cum = []
        for j in range(NL0):
            l0_par_cum[j % 2] += 1
            l0_cum.append(l0_par_cum[j % 2])

        @block.gpsimd
        def _(gpsimd):
            gpsimd.memset(ident[:], 0.0)
            gpsimd.affine_select(
                out=ident[:], in_=ident[:],
                compare_op=mybir.AluOpType.not_equal, fill=1.0, base=0,
                pattern=[[-1, 120]], channel_multiplier=1,
            ).then_inc(init_sem, 1)

        @block.sync
        def _(sync):
            for kind, i in seq:
                if kind == "S":
                    g = i
                    l, s0, S = sgroups[g]
                    if g >= 2:
                        sync.wait_ge(tg_sem, g - 1)
                    src = cls_in[l][:].rearrange("ch h w -> ch (h w)")
                    for t in range(6):
                        sync.dma_start(
                            out=sbuf_s[:, g % 2, t, :S],
                            in_=src[t * 120:(t + 1) * 120, s0:s0 + S],
                        ).then_inc(dma_s[g % 2], 16)
                else:
                    j = i
                    if j >= 2:
                        sync.wait_ge(v0_sem, j - 1)
                    src = cls_in[0][:].rearrange("(a c) h w -> a c (h w)", a=A)[j]
                    src = src.rearrange("c (p f) -> p c f", f=128)  # [128,80,128]
                    sync.dma_start(out=l0buf[:, j % 2, :, :], in_=src).then_inc(
                        dma_l0[j % 2], 16)
            sync.wait_ge(v0_sem, NL0)
            sync.wait_ge(vs_sem, len(vops))
            for l in range(5):
                sync.dma_start(out=m_out[l][:], in_=m_sb[l][:]).then_inc(out_sem, 16)
            sync.wait_ge(out_sem, 16 * 5)

        @block.tensor
        def _(tensor):
            tensor.wait_ge(init_sem, 1)
            bid0 = 0
            cur_vs = 0
            for g, (l, s0, S) in enumerate(sgroups):
                tensor.wait_ge(dma_s[g % 2], 16 * s_cum[g])
                for bi in range(sg_blocks[g]):
                    b = bid0 + bi
                    w = blocks[b][4]
                    need = vs_need(b)
                    if need > cur_vs:
                        tensor.wait_ge(vs_sem, need)
                        cur_vs = need
                    p = b % 4
                    for t in range(6):
                        in_ = sbuf_s[:, g % 2, t, bi * 128:bi * 128 + w]
                        if t < 4:
                            out = psA[:w, p, t * 120:(t + 1) * 120]
                        else:
                            out = psB[:w, p, (t - 4) * 120:(t - 3) * 120]
                        ins = tensor.transpose(out, in_, ident[:])
                        if t == 5:
                            ins.then_inc(t_sem, 1)
                            if bi == sg_blocks[g] - 1:
                                tensor.sem_inc(tg_sem, 1)
                bid0 += sg_blocks[g]

        @block.vector
        def _(vector):
            # vector op order mirrors the DMA interleave: L0 chunk i, then
            # small group i's reduces.
            vop_by_group = [[] for _ in range(NSG)]
            for oi, op in enumerate(vops):
                vop_by_group[blocks[op[0]][0]].append((oi, op))

            def do_l0(j):
                vector.wait_ge(dma_l0[j % 2], 16 * l0_cum[j])
                tile = l0buf[:, j % 2, :, :]  # [128, 80, 128]
                # max tree over classes: 80 -> 40 -> 20 -> 10 -> 5
                s1v = sc1[:].rearrange("p (c f) -> p c f", c=40)
                s2v = sc2[:].rearrange("p (c f) -> p c f", c=20)
                vector.tensor_max(s1v, tile[:, 0:40, :], tile[:, 40:80, :])
                vector.tensor_max(s2v, s1v[:, 0:20, :], s1v[:, 20:40, :])
                s1b = sc1[:, :1280].rearrange("p (c f) -> p c f", c=10)
                vector.tensor_max(s1b, s2v[:, 0:10, :], s2v[:, 10:20, :])
                s2b = sc2[:, :640].rearrange("p (c f) -> p c f", c=5)
                vector.tensor_max(s2b, s1b[:, 0:5, :], s1b[:, 5:10, :])
                # reduce the remaining 5 class-planes (strided innermost)
                vector.reduce_max(
                    m0[:, j * 128:(j + 1) * 128], s2b.transpose([0, 2, 1]),
                    axis=mybir.AxisListType.X).then_inc(v0_sem, 1)

            def do_sop(oi, op):
                nb = len(op)
                b0 = op[0]
                g, bi, l, lb, w = blocks[b0]
                p = b0 % 4
                vector.wait_ge(t_sem, op[-1] + 1)
                # bankA: classes a0-5, bankB: a6-8 (channels a*80+c)
                inA = psA[:w, p:p + nb, :480].rearrange("q b (a c) -> q b a c", c=C)
                inB = psB[:w, p:p + nb, :240].rearrange("q b (a c) -> q b a c", c=C)
                outA = m_sb[l][:w, lb * 9:(lb + nb) * 9].rearrange(
                    "q (b a) -> q b a", a=9)[:, :, 0:6]
                outB = m_sb[l][:w, lb * 9:(lb + nb) * 9].rearrange(
                    "q (b a) -> q b a", a=9)[:, :, 6:9]
                vector.reduce_max(outA, inA, axis=mybir.AxisListType.X)
                vector.reduce_max(outB, inB, axis=mybir.AxisListType.X).then_inc(
                    vs_sem, 1)

            for i in range(max(NSG, NL0)):
                if i < NL0:
                    do_l0(i)
                if i < NSG:
                    for oi, op in vop_by_group[i]:
                        do_sop(oi, op)

    return nc


def _get_graph():
    global _GRAPH
    if _GRAPH is None:
        _GRAPH = _build_graph()
    return _GRAPH


# ---- host-side static tables ----

def _grid_anchors_np(stride, H, W):
    """mmdet AnchorGenerator, float64 then cast f32 (matches reference)."""
    base = stride * 4.0
    ratios = np.array([0.5, 1.0, 2.0], dtype=np.float64)
    scales = np.array([2 ** 0, 2 ** (1.0 / 3), 2 ** (2.0 / 3)], dtype=np.float64)
    h_ratios = np.sqrt(ratios)
    w_ratios = 1.0 / h_ratios
    ws = (base * w_ratios[:, None] * scales[None, :]).reshape(-1)
    hs = (base * h_ratios[:, None] * scales[None, :]).reshape(-1)
    base_anchors = np.stack([-ws / 2, -hs / 2, ws / 2, hs / 2], axis=1)
    sx = np.arange(W, dtype=np.float64) * stride
    sy = np.arange(H, dtype=np.float64) * stride
    yy, xx = np.meshgrid(sy, sx, indexing="ij")
    shifts = np.stack([xx, yy, xx, yy], axis=-1).reshape(-1, 1, 4)
    anchors = (shifts + base_anchors[None]).reshape(-1, 4)
    return anchors.astype(np.float32)


def _lin_index_map(l):
    """linear anchor index n for each element of the device m_{l} layout."""
    P, F = M_SHAPES[l]
    if l == 0:
        p = np.arange(128)[:, None, None]
        a = np.arange(A)[None, :, None]
        f = np.arange(128)[None, None, :]
        s = p * 128 + f
        return ((s * A) + a).reshape(128, 1152)
    q = np.arange(P)[:, None]
    col = np.arange(F)[None, :]
    lb = col // A
    a = col % A
    s = lb * 128 + q
    return (s * A) + a  # [P, F]


_ANCHORS = [_grid_anchors_np(STRIDES[l], SIZES[l], SIZES[l]) for l in range(5)]
_LINMAP = [_lin_index_map(l) for l in range(5)]


def _decode_np(anchors, deltas):
    dx, dy = deltas[..., 0], deltas[..., 1]
    dw = np.clip(deltas[..., 2], -MAX_RATIO, MAX_RATIO).astype(np.float32)
    dh = np.clip(deltas[..., 3], -MAX_RATIO, MAX_RATIO).astype(np.float32)
    px = (anchors[..., 0] + anchors[..., 2]) * np.float32(0.5)
    py = (anchors[..., 1] + anchors[..., 3]) * np.float32(0.5)
    pw = anchors[..., 2] - anchors[..., 0]
    ph = anchors[..., 3] - anchors[..., 1]
    gx = px + pw * dx
    gy = py + ph * dy
    gw = pw * np.exp(dw)
    gh = ph * np.exp(dh)
    half = np.float32(0.5)
    x1 = np.clip(gx - gw * half, 0.0, IMG).astype(np.float32)
    y1 = np.clip(gy - gh * half, 0.0, IMG).astype(np.float32)
    x2 = np.clip(gx + gw * half, 0.0, IMG).astype(np.float32)
    y2 = np.clip(gy + gh * half, 0.0, IMG).astype(np.float32)
    return np.stack([x1, y1, x2, y2], axis=-1)


def _run_device(in_maps, **kw):
    nc = _get_graph()
    return run_bass_kernel_spmd(nc, in_maps, core_ids=list(range(B)), **kw)


def kernel(**inputs):
    cls = [np.asarray(inputs[f"cls_score_{l}"], dtype=np.float32) for l in range(5)]
    box = [np.asarray(inputs[f"bbox_pred_{l}"], dtype=np.float32) for l in range(5)]

    in_maps = [
        {f"cls_score_{l}": np.ascontiguousarray(cls[l][b]) for l in range(5)}
        for b in range(B)
    ]
    res = _run_device(in_maps).results

    bboxes = np.empty((B, 5 * NMS_PRE, 4), dtype=np.float32)
    scores = np.empty((B, 5 * NMS_PRE, C), dtype=np.float32)

    for b in range(B):
        col = 0
        for l in range(5):
            H = SIZES[l]
            m_dev = res[b][f"m_{l}"]
            vals = m_dev.ravel()
            lins = _LINMAP[l].ravel()
            # top-500 by value desc, ties -> lowest linear index (lax.top_k)
            if vals.size > 2 * NMS_PRE:
                part = np.argpartition(-vals, NMS_PRE + 16)[: NMS_PRE + 16]
            else:
                part = np.arange(vals.size)
            pv = vals[part]
            pl = lins[part]
            order = np.lexsort((pl, -pv))[:NMS_PRE]
            sel_lin = pl[order]  # [500] linear anchor idx, sorted desc by score

            a = sel_lin % A
            s = sel_lin // A
            h = s // H
            w = s % H
            # scores: sigmoid of the 80 logits of each selected anchor
            logit_rows = cls[l][b][(a[:, None] * C + np.arange(C)[None, :]), h[:, None], w[:, None]]
            scores[b, col:col + NMS_PRE] = 1.0 / (1.0 + np.exp(-logit_rows))
            # deltas + anchors -> decode
            d = box[l][b][(a[:, None] * 4 + np.arange(4)[None, :]), h[:, None], w[:, None]]
            anc = _ANCHORS[l][sel_lin]
            bboxes[b, col:col + NMS_PRE] = _decode_np(anc, d)
            col += NMS_PRE

    return bboxes, scores
